# revision 10
# baseline (speedup 1.0000x reference)
"""Trainium2 Bass kernel for nn_BranchingLayer (gnn_message_passing), v8.

Computation (reference):
    parents_ftxs = x[idxs_level]                      # identity gather (arange)
    pg           = global_features[parents_idxs % B]  # random gather
    h1 = leaky_relu([parents_ftxs, pg] @ W1 + b1)
    h2 = h1 @ W2 + b2 + repeat(parents_ftxs, 2, -1)
    children = interleave-reshape(h2)                 # child (p,br,b) feat f = h2[p*B+b, br*128+f]
    out = concat([x, children])

v8 design (8 cores, 32768 rows/core), transposed dataflow + weight jamming:
  - host sends xT [128, R] bf16 and duplicated pgT [128, R] bf16.
  - Work unit is a SUPER-GROUP of 1024 rows = two 512-col halves (a, b).
    Every stationary weight is streamed over both halves back-to-back and the
    duplicate LDWEIGHTS for the second half is deleted post-hoc
    (_dedupe_ldweights), so the PE skips the ~46ns weight-swap drain on half
    the matmuls (trace v7: 2068ns/group -> target ~1650ns/group).
  - mm1: h1T = W1x^T xT + W1g^T pgT (N=512 per pass); the two K=64 pg passes
    run concurrently via PE row tiling ((0,0)/(64,0)), quadrant weights are
    loaded once per super-group.
  - ACT lrelu (+b1 per partition) -> 4x [128,512] bf16 tiles per super-group.
  - mm2 with the v7 channel permutation: half m, partition p computes channel
    2p+m (host shuffles W2 columns). psum2[m] is one [128,1024] tile (2 banks,
    halves a|b), so the residual add + PSUM->SBUF bf16 runs as a single
    [128,1024] DVE tensor_tensor per m half (amortizes the ~270cy DVE
    overhead; residual xT slice is naturally contiguous).
  - input loads on the GpSimd SWDGE queue (Scalar stays ACT-only), stores on
    Sync HWDGE per super-group; no PE warmup matmuls (the ~4us serial warmup
    delay outweighed the HAM cold penalty).
"""

import sys

import numpy as np

try:
    import ml_dtypes
except ImportError:
    ml_dtypes = None

if "/opt/trn_rl_repo" not in sys.path:
    sys.path.insert(0, "/opt/trn_rl_repo")

N_PARENTS = 256
BATCH = 1024
N_FEAT = 128
N_BR = 2
N_GLOBAL = 64
N_CORES = 8
ROWS = N_PARENTS * BATCH            # 262144
RPC = ROWS // N_CORES               # 32768 rows per core
GROUP = 512                         # rows per matmul pass (one PSUM bank)
SG = 2 * GROUP                      # super-group: weight-jammed pair
HID = 256
N_WARMUP = 6                         # HAM warmup matmuls (N=512 each)

CHUNK_SIZES = [1024, 1024] + [2048] * 14 + [1024, 1024]
assert sum(CHUNK_SIZES) == RPC
N_CHUNKS = len(CHUNK_SIZES)
CHUNK_OFFS = [sum(CHUNK_SIZES[:i]) for i in range(N_CHUNKS)]
SGS = []                            # global super-group -> (chunk, sg-in-chunk)
for _c, _sz in enumerate(CHUNK_SIZES):
    for _si in range(_sz // SG):
        SGS.append((_c, _si))
NSG = len(SGS)                      # 32
PREFETCH = 3                        # chunks in flight beyond the current one

_CACHE = {}


def _split_multiwait(nc, mybir):
    """This image's walrus accepts only one sync-wait per instruction; hoist
    extra waits onto same-engine NOPs inserted before the instruction."""
    for f in nc.m.functions:
        for bb in f.blocks:
            new_insts = []
            changed = False
            for inst in bb.instructions:
                si = inst.sync_info
                if si is not None and len(si.on_wait) > 1:
                    waits = list(si.on_wait)
                    for w in waits[:-1]:
                        new_insts.append(
                            mybir.InstNoOp(
                                name=nc.get_next_instruction_name(),
                                engine=inst.engine,
                                sync_info=mybir.SyncInfo(on_wait=[w], on_update=[]),
                            )
                        )
                    inst.sync_info = mybir.SyncInfo(
                        on_wait=[waits[-1]], on_update=list(si.on_update)
                    )
                    changed = True
                new_insts.append(inst)
            if changed:
                bb.instructions = new_insts


def _dedupe_ldweights(nc, mybir):
    """Delete InstLdweights that reload the PE row-region with weights that
    are already resident (weight jamming). Matmuls here are non-self-loading
    (ldweights=False), so a deleted duplicate load leaves the prior identical
    weights in the array. Waits on a deleted load are preserved on a NOP."""
    pe = mybir.EngineType.PE
    keep_state = ("InstNoOp", "InstEventSemaphore", "InstMatmult")
    for f in nc.m.functions:
        for bb in f.blocks:
            out = []
            loaded = {}  # row0 -> (nrows, signature)
            for inst in bb.instructions:
                if inst.engine != pe:
                    out.append(inst)
                    continue
                tn = type(inst).__name__
                if tn == "InstLdweights":
                    tp = inst.tile_position
                    row0 = tp[0] if tp else 0
                    ts_ = inst.tile_size
                    nrows = ts_[0] if ts_ else 128
                    sig = (
                        nrows,
                        str(inst.ins),
                        str(tp),
                        str(ts_),
                        str(getattr(inst, "perf_mode", None)),
                        str(getattr(inst, "is_transpose", None)),
                    )
                    if loaded.get(row0) == sig:
                        si = inst.sync_info
                        if si is not None and (si.on_wait or si.on_update):
                            out.append(
                                mybir.InstNoOp(
                                    name=nc.get_next_instruction_name(),
                                    engine=pe,
                                    sync_info=si,
                                )
                            )
                        continue
                    for r0 in list(loaded):
                        n0 = loaded[r0][0]
                        if not (r0 + n0 <= row0 or row0 + nrows <= r0):
                            del loaded[r0]
                    loaded[row0] = sig
                    out.append(inst)
                else:
                    if tn == "InstMatmult":
                        if inst.ldweights:
                            loaded.clear()
                    elif tn not in keep_state:
                        loaded.clear()
                    out.append(inst)
            bb.instructions = out


def _build_program():
    key = "prog_v8"
    if key in _CACHE:
        return _CACHE[key]

    import concourse.bass as bass
    import concourse.mybir as mybir
    import concourse.tile as tile

    f32 = mybir.dt.float32
    bf16 = mybir.dt.bfloat16

    nc = bass.Bass()
    xt = nc.declare_dram_parameter("xt", [N_FEAT, RPC], bf16, isOutput=False)
    pgt = nc.declare_dram_parameter("pgt", [128, RPC], bf16, isOutput=False)
    # packed weights: [0:256]=w1x, [256:384]=w1g, [384:640]=w2a, [640:896]=w2b
    wpk = nc.declare_dram_parameter("wpk", [128, 896], bf16, isOutput=False)
    b1c = nc.declare_dram_parameter("b1c", [128, 2], f32, isOutput=False)
    cht = nc.declare_dram_parameter("cht", [HID, RPC], bf16, isOutput=True)

    AF = mybir.ActivationFunctionType

    with tile.TileContext(nc) as tc:
        with (
            tc.tile_pool(name="const", bufs=1) as cpool,
            tc.tile_pool(name="xin", bufs=5) as xpool,
            tc.tile_pool(name="pg", bufs=5) as gpool,
            tc.tile_pool(name="h1", bufs=3) as hpool,
            tc.tile_pool(name="cout", bufs=5) as opool,
            tc.tile_pool(name="ps1", bufs=1, space="PSUM") as ps1,
            tc.tile_pool(name="ps2", bufs=1, space="PSUM") as ps2,
        ):
            # ---- HAM warmup: the PE idles ~7-11us waiting for the first
            # loads anyway (runtime preamble + DMA completion lag), so free
            # dummy matmuls there start the 3.4us HAM activity window early
            # and the first real matmuls run at 2.4 GHz instead of 1.2.
            wsc = cpool.tile([128, GROUP], bf16, tag="warm_sc")
            nc.gpsimd.memset(wsc[:, :], 0.0)
            wps = ps1.tile([128, GROUP], f32, tag="p1_00", name="warm_ps")
            for i in range(N_WARMUP):
                nc.tensor.matmul(wps[:, :], wsc[:, 0:128], wsc[:, :],
                                 start=True, stop=True)

            # ---- weights: one packed DMA on Sync; b1 separately
            wpks = cpool.tile([128, 896], bf16)
            nc.sync.dma_start(wpks[:], wpk[:])
            b1s = cpool.tile([128, 2], f32)
            nc.sync.dma_start(b1s[:], b1c[:])
            w1xs = wpks[:, 0:256]
            w1gs = wpks[:, 256:384]
            w2a = wpks[:, 384:640]
            w2b = wpks[:, 640:896]

            chunks = []  # per-chunk input tiles: (xt_t, pg_t)

            def load_chunk(c, pg_engine=None):
                off, csz = CHUNK_OFFS[c], CHUNK_SIZES[c]
                xt_t = xpool.tile([128, csz], bf16, tag="xt")
                nc.gpsimd.dma_start(xt_t[:, :], xt[:, off:off + csz])
                pg_t = gpool.tile([128, csz], bf16, tag="pg")
                (pg_engine or nc.gpsimd).dma_start(
                    pg_t[:, :], pgt[:, off:off + csz])
                chunks.append((xt_t, pg_t))

            # first chunks' pg loads ride the otherwise-idle Scalar queue so
            # they land before the pipeline needs them (the GpSimd SWDGE
            # issues serially at ~700ns/DMA and would deliver pg1 too late)
            for c in range(min(PREFETCH, N_CHUNKS)):
                load_chunk(c, pg_engine=nc.scalar)

            # preload the Lrelu activation table (first use costs ~2.7us);
            # after the early pg loads so those issue first on Scalar
            wact = cpool.tile([128, 4], bf16, tag="warm_act")
            nc.scalar.activation(wact[:, :], wsc[:, 0:4], AF.Lrelu,
                                 bias=0.0, scale=1.0, alpha=0.01)

            def emit_mm1(G):
                c, si = SGS[G]
                xt_t, pg_t = chunks[c]
                sl = [slice(si * SG + h * GROUP, si * SG + (h + 1) * GROUP)
                      for h in range(2)]
                p1 = [[ps1.tile([128, GROUP], f32, tag=f"p1_{m}{h}",
                                name=f"p1_{m}{h}_{G}") for h in range(2)]
                      for m in range(2)]
                # x passes, weight-jammed over halves
                for m in range(2):
                    for h in range(2):
                        nc.tensor.matmul(
                            p1[m][h][:, :], w1xs[:, m * 128:(m + 1) * 128],
                            xt_t[:, sl[h]], start=True, stop=False,
                        )
                # pg quadrant passes; quadrant weights persist across halves
                for h in range(2):
                    nc.tensor.matmul(
                        p1[0][h][:, :], w1gs[0:64, :], pg_t[0:64, sl[h]],
                        start=False, stop=True, tile_position=(0, 0),
                    )
                    nc.tensor.matmul(
                        p1[1][h][:, :], w1gs[64:128, :], pg_t[64:128, sl[h]],
                        start=False, stop=True, tile_position=(64, 0),
                    )
                return {"G": G, "p1": p1}

            def emit_act(st):
                G = st["G"]
                h1 = [[hpool.tile([128, GROUP], bf16, tag=f"h1_{m}{h}",
                                  name=f"h1_{m}{h}_{G}") for h in range(2)]
                      for m in range(2)]
                for m in range(2):
                    for h in range(2):
                        nc.scalar.activation(
                            h1[m][h][:, :], st["p1"][m][h][:, :], AF.Lrelu,
                            bias=b1s[:, m:m + 1], scale=1.0, alpha=0.01,
                        )
                st["h1"] = h1

            def emit_mm2(st):
                # half m, partition p -> channel 2p+m (host pre-shuffled W2)
                G = st["G"]
                h1 = st["h1"]
                p2 = [ps2.tile([128, SG], f32, tag=f"p2_{m}",
                               name=f"p2_{m}_{G}") for m in range(2)]
                for m in range(2):
                    ws = slice(m * 128, (m + 1) * 128)
                    for h in range(2):
                        nc.tensor.matmul(
                            p2[m][:, h * GROUP:(h + 1) * GROUP], w2a[:, ws],
                            h1[0][h][:, :], start=True, stop=False,
                        )
                    for h in range(2):
                        nc.tensor.matmul(
                            p2[m][:, h * GROUP:(h + 1) * GROUP], w2b[:, ws],
                            h1[1][h][:, :], start=False, stop=True,
                        )
                st["p2"] = p2

            def emit_tail(st):
                # residual add: channel 2p+m needs x[p]; one [128,1024] DVE op
                # per m half, then store this super-group on Sync.
                G = st["G"]
                c, si = SGS[G]
                xt_t, _ = chunks[c]
                sgsl = slice(si * SG, (si + 1) * SG)
                off = CHUNK_OFFS[c] + si * SG
                cho = [opool.tile([128, SG], bf16, tag=f"cho{m}",
                                  name=f"cho{m}_{G}") for m in range(2)]
                for m in range(2):
                    nc.vector.tensor_add(
                        cho[m][:, :], st["p2"][m][:, :], xt_t[:, sgsl])
                    nc.sync.dma_start(
                        cht[m * 128:(m + 1) * 128, off:off + SG], cho[m][:, :])
                if si == CHUNK_SIZES[c] // SG - 1 and c + PREFETCH < N_CHUNKS:
                    load_chunk(c + PREFETCH)

            prev = None
            for G in range(NSG + 1):
                cur = emit_mm1(G) if G < NSG else None
                if prev is not None:
                    emit_mm2(prev)
                if cur is not None:
                    emit_act(cur)
                if prev is not None:
                    emit_tail(prev)
                prev = cur

    _dedupe_ldweights(nc, mybir)
    _split_multiwait(nc, mybir)
    _CACHE[key] = nc
    return nc


def _host_prep(x, global_features, W1, b1, W2, b2, idxs_level, parents_idxs):
    bf = ml_dtypes.bfloat16
    x = np.ascontiguousarray(np.asarray(x, dtype=np.float32))
    G = np.asarray(global_features, dtype=np.float32)
    W1 = np.asarray(W1, dtype=np.float32)
    b1 = np.asarray(b1, dtype=np.float32)
    W2 = np.asarray(W2, dtype=np.float32)
    idxs = np.asarray(idxs_level)
    pidx = np.asarray(parents_idxs)

    if np.array_equal(idxs, np.arange(ROWS, dtype=idxs.dtype)):
        xg = x
    else:  # general gather fallback (host)
        xg = np.ascontiguousarray(x[idxs])

    xt = np.ascontiguousarray(
        xg.reshape(N_CORES, RPC, N_FEAT).transpose(0, 2, 1)).astype(bf)
    pg = G[pidx % BATCH]                              # [ROWS, 64]
    pgt = np.ascontiguousarray(
        pg.reshape(N_CORES, RPC, N_GLOBAL).transpose(0, 2, 1)).astype(bf)
    pgt = np.concatenate([pgt, pgt], axis=1)          # [8, 128, RPC]

    w1xb = W1[0:128, :]                               # [128, 256]
    # rows 0-63: W1g cols 0-127 (hid half 0); rows 64-127: cols 128-255
    w1gb = np.concatenate([W1[128:192, 0:128], W1[128:192, 128:256]], axis=0)
    # channel permutation: half m takes W2 columns m::2 (channel 2p+m at p)
    w2p = np.concatenate([W2[:, 0::2], W2[:, 1::2]], axis=1)  # [256, 256]
    wpk = np.ascontiguousarray(np.concatenate(
        [w1xb, w1gb, w2p[0:128, :], w2p[128:256, :]], axis=1)).astype(bf)
    b1c = np.ascontiguousarray(b1.reshape(2, 128).T)  # [128, 2]

    in_maps = []
    for c in range(N_CORES):
        in_maps.append({
            "xt": xt[c],
            "pgt": pgt[c],
            "wpk": wpk,
            "b1c": b1c,
        })
    return x, in_maps


# channel of cht row (m*128+p) is 2p+m; row holding channel c is (c%2)*128+c//2
_ROW_OF_CHAN = ((np.arange(HID) % 2) * 128 + np.arange(HID) // 2)


def _assemble_core(cht_arr, b2):
    b2 = np.asarray(b2, dtype=np.float32)
    A = np.asarray(cht_arr).astype(np.float32)
    projT = A[_ROW_OF_CHAN, :] + b2[:, None]           # [256 channels, RPC]
    B4 = projT.reshape(2, 128, RPC // BATCH, BATCH)    # [br, f, pp, b]
    return B4.transpose(2, 0, 3, 1).reshape(2 * RPC, N_FEAT)


def _assemble(results, b2):
    return np.concatenate(
        [_assemble_core(results[c]["cht"], b2) for c in range(N_CORES)], axis=0)


def kernel(x, global_features, W1, b1, W2, b2, idxs_level, parents_idxs,
           _trace=False, _trace_kwargs=None):
    from concourse.bass_utils import run_bass_kernel_spmd

    x_np, in_maps = _host_prep(
        x, global_features, W1, b1, W2, b2, idxs_level, parents_idxs
    )
    nc = _build_program()
    res = run_bass_kernel_spmd(
        nc, in_maps, list(range(N_CORES)),
        trace=_trace, **(_trace_kwargs or {}),
    )
    children = _assemble(res.results, b2)
    out = np.concatenate([x_np, children], axis=0)
    if _trace:
        kernel.last_result = res
    return out


# revision 12
# speedup vs baseline: 1.0027x; 1.0027x over previous
"""Trainium2 Bass kernel for nn_BranchingLayer (gnn_message_passing), v8.

Computation (reference):
    parents_ftxs = x[idxs_level]                      # identity gather (arange)
    pg           = global_features[parents_idxs % B]  # random gather
    h1 = leaky_relu([parents_ftxs, pg] @ W1 + b1)
    h2 = h1 @ W2 + b2 + repeat(parents_ftxs, 2, -1)
    children = interleave-reshape(h2)                 # child (p,br,b) feat f = h2[p*B+b, br*128+f]
    out = concat([x, children])

v8 design (8 cores, 32768 rows/core), transposed dataflow + weight jamming:
  - host sends xT [128, R] bf16 and duplicated pgT [128, R] bf16.
  - Work unit is a SUPER-GROUP of 1024 rows = two 512-col halves (a, b).
    Every stationary weight is streamed over both halves back-to-back and the
    duplicate LDWEIGHTS for the second half is deleted post-hoc
    (_dedupe_ldweights), so the PE skips the ~46ns weight-swap drain on half
    the matmuls (trace v7: 2068ns/group -> target ~1650ns/group).
  - mm1: h1T = W1x^T xT + W1g^T pgT (N=512 per pass); the two K=64 pg passes
    run concurrently via PE row tiling ((0,0)/(64,0)), quadrant weights are
    loaded once per super-group.
  - ACT lrelu (+b1 per partition) -> 4x [128,512] bf16 tiles per super-group.
  - mm2 with the v7 channel permutation: half m, partition p computes channel
    2p+m (host shuffles W2 columns). psum2[m] is one [128,1024] tile (2 banks,
    halves a|b), so the residual add + PSUM->SBUF bf16 runs as a single
    [128,1024] DVE tensor_tensor per m half (amortizes the ~270cy DVE
    overhead; residual xT slice is naturally contiguous).
  - input loads on the GpSimd SWDGE queue (Scalar stays ACT-only), stores on
    Sync HWDGE per super-group; no PE warmup matmuls (the ~4us serial warmup
    delay outweighed the HAM cold penalty).
"""

import sys

import numpy as np

try:
    import ml_dtypes
except ImportError:
    ml_dtypes = None

if "/opt/trn_rl_repo" not in sys.path:
    sys.path.insert(0, "/opt/trn_rl_repo")

N_PARENTS = 256
BATCH = 1024
N_FEAT = 128
N_BR = 2
N_GLOBAL = 64
N_CORES = 8
ROWS = N_PARENTS * BATCH            # 262144
RPC = ROWS // N_CORES               # 32768 rows per core
GROUP = 512                         # rows per matmul pass (one PSUM bank)
SG = 2 * GROUP                      # super-group: weight-jammed pair
HID = 256
N_WARMUP = 8                         # HAM warmup matmuls (N=512 each)

CHUNK_SIZES = [1024, 1024] + [2048] * 14 + [1024, 1024]
assert sum(CHUNK_SIZES) == RPC
N_CHUNKS = len(CHUNK_SIZES)
CHUNK_OFFS = [sum(CHUNK_SIZES[:i]) for i in range(N_CHUNKS)]
SGS = []                            # global super-group -> (chunk, sg-in-chunk)
for _c, _sz in enumerate(CHUNK_SIZES):
    for _si in range(_sz // SG):
        SGS.append((_c, _si))
NSG = len(SGS)                      # 32
PREFETCH = 3                        # chunks in flight beyond the current one

_CACHE = {}


def _split_multiwait(nc, mybir):
    """This image's walrus accepts only one sync-wait per instruction; hoist
    extra waits onto same-engine NOPs inserted before the instruction."""
    for f in nc.m.functions:
        for bb in f.blocks:
            new_insts = []
            changed = False
            for inst in bb.instructions:
                si = inst.sync_info
                if si is not None and len(si.on_wait) > 1:
                    waits = list(si.on_wait)
                    for w in waits[:-1]:
                        new_insts.append(
                            mybir.InstNoOp(
                                name=nc.get_next_instruction_name(),
                                engine=inst.engine,
                                sync_info=mybir.SyncInfo(on_wait=[w], on_update=[]),
                            )
                        )
                    inst.sync_info = mybir.SyncInfo(
                        on_wait=[waits[-1]], on_update=list(si.on_update)
                    )
                    changed = True
                new_insts.append(inst)
            if changed:
                bb.instructions = new_insts


def _dedupe_ldweights(nc, mybir):
    """Delete InstLdweights that reload the PE row-region with weights that
    are already resident (weight jamming). Matmuls here are non-self-loading
    (ldweights=False), so a deleted duplicate load leaves the prior identical
    weights in the array. Waits on a deleted load are preserved on a NOP."""
    pe = mybir.EngineType.PE
    keep_state = ("InstNoOp", "InstEventSemaphore", "InstMatmult")
    for f in nc.m.functions:
        for bb in f.blocks:
            out = []
            loaded = {}  # row0 -> (nrows, signature)
            for inst in bb.instructions:
                if inst.engine != pe:
                    out.append(inst)
                    continue
                tn = type(inst).__name__
                if tn == "InstLdweights":
                    tp = inst.tile_position
                    row0 = tp[0] if tp else 0
                    ts_ = inst.tile_size
                    nrows = ts_[0] if ts_ else 128
                    sig = (
                        nrows,
                        str(inst.ins),
                        str(tp),
                        str(ts_),
                        str(getattr(inst, "perf_mode", None)),
                        str(getattr(inst, "is_transpose", None)),
                    )
                    if loaded.get(row0) == sig:
                        si = inst.sync_info
                        if si is not None and (si.on_wait or si.on_update):
                            out.append(
                                mybir.InstNoOp(
                                    name=nc.get_next_instruction_name(),
                                    engine=pe,
                                    sync_info=si,
                                )
                            )
                        continue
                    for r0 in list(loaded):
                        n0 = loaded[r0][0]
                        if not (r0 + n0 <= row0 or row0 + nrows <= r0):
                            del loaded[r0]
                    loaded[row0] = sig
                    out.append(inst)
                else:
                    if tn == "InstMatmult":
                        if inst.ldweights:
                            loaded.clear()
                    elif tn not in keep_state:
                        loaded.clear()
                    out.append(inst)
            bb.instructions = out


def _build_program():
    key = "prog_v8"
    if key in _CACHE:
        return _CACHE[key]

    import concourse.bass as bass
    import concourse.mybir as mybir
    import concourse.tile as tile

    f32 = mybir.dt.float32
    bf16 = mybir.dt.bfloat16

    nc = bass.Bass()
    xt = nc.declare_dram_parameter("xt", [N_FEAT, RPC], bf16, isOutput=False)
    pgt = nc.declare_dram_parameter("pgt", [128, RPC], bf16, isOutput=False)
    # packed weights: [0:256]=w1x, [256:384]=w1g, [384:640]=w2a, [640:896]=w2b
    wpk = nc.declare_dram_parameter("wpk", [128, 896], bf16, isOutput=False)
    b1c = nc.declare_dram_parameter("b1c", [128, 2], f32, isOutput=False)
    cht = nc.declare_dram_parameter("cht", [HID, RPC], bf16, isOutput=True)

    AF = mybir.ActivationFunctionType

    with tile.TileContext(nc) as tc:
        with (
            tc.tile_pool(name="const", bufs=1) as cpool,
            tc.tile_pool(name="xin", bufs=5) as xpool,
            tc.tile_pool(name="pg", bufs=5) as gpool,
            tc.tile_pool(name="h1", bufs=3) as hpool,
            tc.tile_pool(name="cout", bufs=5) as opool,
            tc.tile_pool(name="ps1", bufs=1, space="PSUM") as ps1,
            tc.tile_pool(name="ps2", bufs=1, space="PSUM") as ps2,
        ):
            # ---- HAM warmup: the PE idles ~7-11us waiting for the first
            # loads anyway (runtime preamble + DMA completion lag), so free
            # dummy matmuls there start the 3.4us HAM activity window early
            # and the first real matmuls run at 2.4 GHz instead of 1.2.
            wsc = cpool.tile([128, GROUP], bf16, tag="warm_sc")
            nc.gpsimd.memset(wsc[:, :], 0.0)
            wps = ps1.tile([128, GROUP], f32, tag="p1_00", name="warm_ps")
            for i in range(N_WARMUP):
                nc.tensor.matmul(wps[:, :], wsc[:, 0:128], wsc[:, :],
                                 start=True, stop=True)

            # ---- weights: one packed DMA on Sync; b1 separately
            wpks = cpool.tile([128, 896], bf16)
            nc.sync.dma_start(wpks[:], wpk[:])
            b1s = cpool.tile([128, 2], f32)
            nc.sync.dma_start(b1s[:], b1c[:])
            w1xs = wpks[:, 0:256]
            w1gs = wpks[:, 256:384]
            w2a = wpks[:, 384:640]
            w2b = wpks[:, 640:896]

            chunks = []  # per-chunk input tiles: (xt_t, pg_t)

            def load_chunk(c, pg_engine=None):
                off, csz = CHUNK_OFFS[c], CHUNK_SIZES[c]
                xt_t = xpool.tile([128, csz], bf16, tag="xt")
                nc.gpsimd.dma_start(xt_t[:, :], xt[:, off:off + csz])
                pg_t = gpool.tile([128, csz], bf16, tag="pg")
                (pg_engine or nc.gpsimd).dma_start(
                    pg_t[:, :], pgt[:, off:off + csz])
                chunks.append((xt_t, pg_t))

            # chunk0's pg rides Sync (right after the weights+b1, which the
            # first matmul gates on anyway) so the first super-group's pg
            # quadrant passes don't stall on the serial GpSimd SWDGE issue;
            # everything else stays on GpSimd in criticality order.
            load_chunk(0, pg_engine=nc.sync)
            for c in range(1, min(PREFETCH, N_CHUNKS)):
                load_chunk(c)

            # preload the Lrelu activation table (first use costs ~2.7us)
            wact = cpool.tile([128, 4], bf16, tag="warm_act")
            nc.scalar.activation(wact[:, :], wsc[:, 0:4], AF.Lrelu,
                                 bias=0.0, scale=1.0, alpha=0.01)

            def emit_mm1(G):
                c, si = SGS[G]
                xt_t, pg_t = chunks[c]
                sl = [slice(si * SG + h * GROUP, si * SG + (h + 1) * GROUP)
                      for h in range(2)]
                p1 = [[ps1.tile([128, GROUP], f32, tag=f"p1_{m}{h}",
                                name=f"p1_{m}{h}_{G}") for h in range(2)]
                      for m in range(2)]
                # x passes, weight-jammed over halves
                for m in range(2):
                    for h in range(2):
                        nc.tensor.matmul(
                            p1[m][h][:, :], w1xs[:, m * 128:(m + 1) * 128],
                            xt_t[:, sl[h]], start=True, stop=False,
                        )
                # pg quadrant passes; quadrant weights persist across halves
                for h in range(2):
                    nc.tensor.matmul(
                        p1[0][h][:, :], w1gs[0:64, :], pg_t[0:64, sl[h]],
                        start=False, stop=True, tile_position=(0, 0),
                    )
                    nc.tensor.matmul(
                        p1[1][h][:, :], w1gs[64:128, :], pg_t[64:128, sl[h]],
                        start=False, stop=True, tile_position=(64, 0),
                    )
                return {"G": G, "p1": p1}

            def emit_act(st):
                G = st["G"]
                h1 = [[hpool.tile([128, GROUP], bf16, tag=f"h1_{m}{h}",
                                  name=f"h1_{m}{h}_{G}") for h in range(2)]
                      for m in range(2)]
                for m in range(2):
                    for h in range(2):
                        nc.scalar.activation(
                            h1[m][h][:, :], st["p1"][m][h][:, :], AF.Lrelu,
                            bias=b1s[:, m:m + 1], scale=1.0, alpha=0.01,
                        )
                st["h1"] = h1

            def emit_mm2(st):
                # half m, partition p -> channel 2p+m (host pre-shuffled W2)
                G = st["G"]
                h1 = st["h1"]
                p2 = [ps2.tile([128, SG], f32, tag=f"p2_{m}",
                               name=f"p2_{m}_{G}") for m in range(2)]
                for m in range(2):
                    ws = slice(m * 128, (m + 1) * 128)
                    for h in range(2):
                        nc.tensor.matmul(
                            p2[m][:, h * GROUP:(h + 1) * GROUP], w2a[:, ws],
                            h1[0][h][:, :], start=True, stop=False,
                        )
                    for h in range(2):
                        nc.tensor.matmul(
                            p2[m][:, h * GROUP:(h + 1) * GROUP], w2b[:, ws],
                            h1[1][h][:, :], start=False, stop=True,
                        )
                st["p2"] = p2

            def emit_tail(st):
                # residual add: channel 2p+m needs x[p]; one [128,1024] DVE op
                # per m half, then store this super-group on Sync.
                G = st["G"]
                c, si = SGS[G]
                xt_t, _ = chunks[c]
                sgsl = slice(si * SG, (si + 1) * SG)
                off = CHUNK_OFFS[c] + si * SG
                cho = [opool.tile([128, SG], bf16, tag=f"cho{m}",
                                  name=f"cho{m}_{G}") for m in range(2)]
                for m in range(2):
                    nc.vector.tensor_add(
                        cho[m][:, :], st["p2"][m][:, :], xt_t[:, sgsl])
                    nc.sync.dma_start(
                        cht[m * 128:(m + 1) * 128, off:off + SG], cho[m][:, :])
                if si == CHUNK_SIZES[c] // SG - 1 and c + PREFETCH < N_CHUNKS:
                    load_chunk(c + PREFETCH)

            prev = None
            for G in range(NSG + 1):
                cur = emit_mm1(G) if G < NSG else None
                if prev is not None:
                    emit_mm2(prev)
                if cur is not None:
                    emit_act(cur)
                if prev is not None:
                    emit_tail(prev)
                prev = cur

    _dedupe_ldweights(nc, mybir)
    _split_multiwait(nc, mybir)
    _CACHE[key] = nc
    return nc


def _host_prep(x, global_features, W1, b1, W2, b2, idxs_level, parents_idxs):
    bf = ml_dtypes.bfloat16
    x = np.ascontiguousarray(np.asarray(x, dtype=np.float32))
    G = np.asarray(global_features, dtype=np.float32)
    W1 = np.asarray(W1, dtype=np.float32)
    b1 = np.asarray(b1, dtype=np.float32)
    W2 = np.asarray(W2, dtype=np.float32)
    idxs = np.asarray(idxs_level)
    pidx = np.asarray(parents_idxs)

    if np.array_equal(idxs, np.arange(ROWS, dtype=idxs.dtype)):
        xg = x
    else:  # general gather fallback (host)
        xg = np.ascontiguousarray(x[idxs])

    xt = np.ascontiguousarray(
        xg.reshape(N_CORES, RPC, N_FEAT).transpose(0, 2, 1)).astype(bf)
    pg = G[pidx % BATCH]                              # [ROWS, 64]
    pgt = np.ascontiguousarray(
        pg.reshape(N_CORES, RPC, N_GLOBAL).transpose(0, 2, 1)).astype(bf)
    pgt = np.concatenate([pgt, pgt], axis=1)          # [8, 128, RPC]

    w1xb = W1[0:128, :]                               # [128, 256]
    # rows 0-63: W1g cols 0-127 (hid half 0); rows 64-127: cols 128-255
    w1gb = np.concatenate([W1[128:192, 0:128], W1[128:192, 128:256]], axis=0)
    # channel permutation: half m takes W2 columns m::2 (channel 2p+m at p)
    w2p = np.concatenate([W2[:, 0::2], W2[:, 1::2]], axis=1)  # [256, 256]
    wpk = np.ascontiguousarray(np.concatenate(
        [w1xb, w1gb, w2p[0:128, :], w2p[128:256, :]], axis=1)).astype(bf)
    b1c = np.ascontiguousarray(b1.reshape(2, 128).T)  # [128, 2]

    in_maps = []
    for c in range(N_CORES):
        in_maps.append({
            "xt": xt[c],
            "pgt": pgt[c],
            "wpk": wpk,
            "b1c": b1c,
        })
    return x, in_maps


# channel of cht row (m*128+p) is 2p+m; row holding channel c is (c%2)*128+c//2
_ROW_OF_CHAN = ((np.arange(HID) % 2) * 128 + np.arange(HID) // 2)


def _assemble_core(cht_arr, b2):
    b2 = np.asarray(b2, dtype=np.float32)
    A = np.asarray(cht_arr).astype(np.float32)
    projT = A[_ROW_OF_CHAN, :] + b2[:, None]           # [256 channels, RPC]
    B4 = projT.reshape(2, 128, RPC // BATCH, BATCH)    # [br, f, pp, b]
    return B4.transpose(2, 0, 3, 1).reshape(2 * RPC, N_FEAT)


def _assemble(results, b2):
    return np.concatenate(
        [_assemble_core(results[c]["cht"], b2) for c in range(N_CORES)], axis=0)


def kernel(x, global_features, W1, b1, W2, b2, idxs_level, parents_idxs,
           _trace=False, _trace_kwargs=None):
    from concourse.bass_utils import run_bass_kernel_spmd

    x_np, in_maps = _host_prep(
        x, global_features, W1, b1, W2, b2, idxs_level, parents_idxs
    )
    nc = _build_program()
    res = run_bass_kernel_spmd(
        nc, in_maps, list(range(N_CORES)),
        trace=_trace, **(_trace_kwargs or {}),
    )
    children = _assemble(res.results, b2)
    out = np.concatenate([x_np, children], axis=0)
    if _trace:
        kernel.last_result = res
    return out


# revision 13
# speedup vs baseline: 1.0146x; 1.0119x over previous
"""Trainium2 Bass kernel for nn_BranchingLayer (gnn_message_passing), v8.

Computation (reference):
    parents_ftxs = x[idxs_level]                      # identity gather (arange)
    pg           = global_features[parents_idxs % B]  # random gather
    h1 = leaky_relu([parents_ftxs, pg] @ W1 + b1)
    h2 = h1 @ W2 + b2 + repeat(parents_ftxs, 2, -1)
    children = interleave-reshape(h2)                 # child (p,br,b) feat f = h2[p*B+b, br*128+f]
    out = concat([x, children])

v8 design (8 cores, 32768 rows/core), transposed dataflow + weight jamming:
  - host sends xT [128, R] bf16 and duplicated pgT [128, R] bf16.
  - Work unit is a SUPER-GROUP of 1024 rows = two 512-col halves (a, b).
    Every stationary weight is streamed over both halves back-to-back and the
    duplicate LDWEIGHTS for the second half is deleted post-hoc
    (_dedupe_ldweights), so the PE skips the ~46ns weight-swap drain on half
    the matmuls (trace v7: 2068ns/group -> target ~1650ns/group).
  - mm1: h1T = W1x^T xT + W1g^T pgT (N=512 per pass); the two K=64 pg passes
    run concurrently via PE row tiling ((0,0)/(64,0)), quadrant weights are
    loaded once per super-group.
  - ACT lrelu (+b1 per partition) -> 4x [128,512] bf16 tiles per super-group.
  - mm2 with the v7 channel permutation: half m, partition p computes channel
    2p+m (host shuffles W2 columns). psum2[m] is one [128,1024] tile (2 banks,
    halves a|b), so the residual add + PSUM->SBUF bf16 runs as a single
    [128,1024] DVE tensor_tensor per m half (amortizes the ~270cy DVE
    overhead; residual xT slice is naturally contiguous).
  - input loads on the GpSimd SWDGE queue (Scalar stays ACT-only), stores on
    Sync HWDGE per super-group; no PE warmup matmuls (the ~4us serial warmup
    delay outweighed the HAM cold penalty).
"""

import sys

import numpy as np

try:
    import ml_dtypes
except ImportError:
    ml_dtypes = None

if "/opt/trn_rl_repo" not in sys.path:
    sys.path.insert(0, "/opt/trn_rl_repo")

N_PARENTS = 256
BATCH = 1024
N_FEAT = 128
N_BR = 2
N_GLOBAL = 64
N_CORES = 8
ROWS = N_PARENTS * BATCH            # 262144
RPC = ROWS // N_CORES               # 32768 rows per core
GROUP = 512                         # rows per matmul pass (one PSUM bank)
SG = 2 * GROUP                      # super-group: weight-jammed pair
HID = 256
N_WARMUP = 8                         # HAM warmup matmuls (N=512 each)

CHUNK_SIZES = [1024, 1024] + [2048] * 14 + [1024, 1024]
assert sum(CHUNK_SIZES) == RPC
N_CHUNKS = len(CHUNK_SIZES)
CHUNK_OFFS = [sum(CHUNK_SIZES[:i]) for i in range(N_CHUNKS)]
SGS = []                            # global super-group -> (chunk, sg-in-chunk)
for _c, _sz in enumerate(CHUNK_SIZES):
    for _si in range(_sz // SG):
        SGS.append((_c, _si))
NSG = len(SGS)                      # 32
PREFETCH = 3                        # chunks in flight beyond the current one

_CACHE = {}


def _split_multiwait(nc, mybir):
    """This image's walrus accepts only one sync-wait per instruction; hoist
    extra waits onto same-engine NOPs inserted before the instruction."""
    for f in nc.m.functions:
        for bb in f.blocks:
            new_insts = []
            changed = False
            for inst in bb.instructions:
                si = inst.sync_info
                if si is not None and len(si.on_wait) > 1:
                    waits = list(si.on_wait)
                    for w in waits[:-1]:
                        new_insts.append(
                            mybir.InstNoOp(
                                name=nc.get_next_instruction_name(),
                                engine=inst.engine,
                                sync_info=mybir.SyncInfo(on_wait=[w], on_update=[]),
                            )
                        )
                    inst.sync_info = mybir.SyncInfo(
                        on_wait=[waits[-1]], on_update=list(si.on_update)
                    )
                    changed = True
                new_insts.append(inst)
            if changed:
                bb.instructions = new_insts


def _dedupe_ldweights(nc, mybir):
    """Delete InstLdweights that reload the PE row-region with weights that
    are already resident (weight jamming). Matmuls here are non-self-loading
    (ldweights=False), so a deleted duplicate load leaves the prior identical
    weights in the array. Waits on a deleted load are preserved on a NOP."""
    pe = mybir.EngineType.PE
    keep_state = ("InstNoOp", "InstEventSemaphore", "InstMatmult")
    for f in nc.m.functions:
        for bb in f.blocks:
            out = []
            loaded = {}  # row0 -> (nrows, signature)
            for inst in bb.instructions:
                if inst.engine != pe:
                    out.append(inst)
                    continue
                tn = type(inst).__name__
                if tn == "InstLdweights":
                    tp = inst.tile_position
                    row0 = tp[0] if tp else 0
                    ts_ = inst.tile_size
                    nrows = ts_[0] if ts_ else 128
                    sig = (
                        nrows,
                        str(inst.ins),
                        str(tp),
                        str(ts_),
                        str(getattr(inst, "perf_mode", None)),
                        str(getattr(inst, "is_transpose", None)),
                    )
                    if loaded.get(row0) == sig:
                        si = inst.sync_info
                        if si is not None and (si.on_wait or si.on_update):
                            out.append(
                                mybir.InstNoOp(
                                    name=nc.get_next_instruction_name(),
                                    engine=pe,
                                    sync_info=si,
                                )
                            )
                        continue
                    for r0 in list(loaded):
                        n0 = loaded[r0][0]
                        if not (r0 + n0 <= row0 or row0 + nrows <= r0):
                            del loaded[r0]
                    loaded[row0] = sig
                    out.append(inst)
                else:
                    if tn == "InstMatmult":
                        if inst.ldweights:
                            loaded.clear()
                    elif tn not in keep_state:
                        loaded.clear()
                    out.append(inst)
            bb.instructions = out


def _build_program():
    key = "prog_v8"
    if key in _CACHE:
        return _CACHE[key]

    import concourse.bass as bass
    import concourse.mybir as mybir
    import concourse.tile as tile

    f32 = mybir.dt.float32
    bf16 = mybir.dt.bfloat16

    nc = bass.Bass()
    xt = nc.declare_dram_parameter("xt", [N_FEAT, RPC], bf16, isOutput=False)
    pgt = nc.declare_dram_parameter("pgt", [128, RPC], bf16, isOutput=False)
    # packed weights: [0:256]=w1x, [256:384]=w1g, [384:640]=w2a, [640:896]=w2b
    wpk = nc.declare_dram_parameter("wpk", [128, 896], bf16, isOutput=False)
    b1c = nc.declare_dram_parameter("b1c", [128, 2], f32, isOutput=False)
    cht = nc.declare_dram_parameter("cht", [HID, RPC], bf16, isOutput=True)

    AF = mybir.ActivationFunctionType

    with tile.TileContext(nc) as tc:
        with (
            tc.tile_pool(name="const", bufs=1) as cpool,
            tc.tile_pool(name="xin", bufs=5) as xpool,
            tc.tile_pool(name="pg", bufs=5) as gpool,
            tc.tile_pool(name="h1", bufs=3) as hpool,
            tc.tile_pool(name="cout", bufs=5) as opool,
            tc.tile_pool(name="ps1", bufs=1, space="PSUM") as ps1,
            tc.tile_pool(name="ps2", bufs=1, space="PSUM") as ps2,
        ):
            # ---- HAM warmup: the PE idles ~7-11us waiting for the first
            # loads anyway (runtime preamble + DMA completion lag), so free
            # dummy matmuls there start the 3.4us HAM activity window early
            # and the first real matmuls run at 2.4 GHz instead of 1.2.
            wsc = cpool.tile([128, GROUP], bf16, tag="warm_sc")
            nc.gpsimd.memset(wsc[:, :], 0.0)
            wps = ps1.tile([128, GROUP], f32, tag="p1_00", name="warm_ps")
            for i in range(N_WARMUP):
                nc.tensor.matmul(wps[:, :], wsc[:, 0:128], wsc[:, :],
                                 start=True, stop=True)

            # ---- weights: one packed DMA on Sync; b1 separately
            wpks = cpool.tile([128, 896], bf16)
            nc.sync.dma_start(wpks[:], wpk[:])
            b1s = cpool.tile([128, 2], f32)
            nc.sync.dma_start(b1s[:], b1c[:])
            w1xs = wpks[:, 0:256]
            w1gs = wpks[:, 256:384]
            w2a = wpks[:, 384:640]
            w2b = wpks[:, 640:896]

            chunks = []  # per-chunk input tiles: (xt_t, pg_t)

            def load_chunk(c, pg_engine=None):
                off, csz = CHUNK_OFFS[c], CHUNK_SIZES[c]
                xt_t = xpool.tile([128, csz], bf16, tag="xt")
                nc.gpsimd.dma_start(xt_t[:, :], xt[:, off:off + csz])
                pg_t = gpool.tile([128, csz], bf16, tag="pg")
                (pg_engine or nc.gpsimd).dma_start(
                    pg_t[:, :], pgt[:, off:off + csz])
                chunks.append((xt_t, pg_t))

            # chunk0's pg rides Sync (right after the weights+b1, which the
            # first matmul gates on anyway) so the first super-group's pg
            # quadrant passes don't stall on the serial GpSimd SWDGE issue;
            # everything else stays on GpSimd in criticality order.
            load_chunk(0, pg_engine=nc.sync)
            for c in range(1, min(PREFETCH, N_CHUNKS)):
                load_chunk(c)

            # preload the Lrelu activation table (first use costs ~2.7us)
            wact = cpool.tile([128, 4], bf16, tag="warm_act")
            nc.scalar.activation(wact[:, :], wsc[:, 0:4], AF.Lrelu,
                                 bias=0.0, scale=1.0, alpha=0.01)

            def emit_mm1(G):
                c, si = SGS[G]
                xt_t, pg_t = chunks[c]
                sl = [slice(si * SG + h * GROUP, si * SG + (h + 1) * GROUP)
                      for h in range(2)]
                p1 = [[ps1.tile([128, GROUP], f32, tag=f"p1_{m}{h}",
                                name=f"p1_{m}{h}_{G}") for h in range(2)]
                      for m in range(2)]
                # x passes, weight-jammed over halves
                for m in range(2):
                    for h in range(2):
                        nc.tensor.matmul(
                            p1[m][h][:, :], w1xs[:, m * 128:(m + 1) * 128],
                            xt_t[:, sl[h]], start=True, stop=False,
                        )
                # pg quadrant passes; quadrant weights persist across halves
                for h in range(2):
                    nc.tensor.matmul(
                        p1[0][h][:, :], w1gs[0:64, :], pg_t[0:64, sl[h]],
                        start=False, stop=True, tile_position=(0, 0),
                    )
                    nc.tensor.matmul(
                        p1[1][h][:, :], w1gs[64:128, :], pg_t[64:128, sl[h]],
                        start=False, stop=True, tile_position=(64, 0),
                    )
                return {"G": G, "p1": p1}

            def emit_act(st):
                G = st["G"]
                h1 = [[hpool.tile([128, GROUP], bf16, tag=f"h1_{m}{h}",
                                  name=f"h1_{m}{h}_{G}") for h in range(2)]
                      for m in range(2)]
                # h-major order: ACT(m1,h0) lands 2nd instead of 3rd, tripling
                # the slack on mm1(G+1)'s x-m1 reuse of the p1_1x psum banks
                for h in range(2):
                    for m in range(2):
                        nc.scalar.activation(
                            h1[m][h][:, :], st["p1"][m][h][:, :], AF.Lrelu,
                            bias=b1s[:, m:m + 1], scale=1.0, alpha=0.01,
                        )
                st["h1"] = h1

            def emit_mm2(st):
                # half m, partition p -> channel 2p+m (host pre-shuffled W2)
                G = st["G"]
                h1 = st["h1"]
                p2 = [ps2.tile([128, SG], f32, tag=f"p2_{m}",
                               name=f"p2_{m}_{G}") for m in range(2)]
                for m in range(2):
                    ws = slice(m * 128, (m + 1) * 128)
                    for h in range(2):
                        nc.tensor.matmul(
                            p2[m][:, h * GROUP:(h + 1) * GROUP], w2a[:, ws],
                            h1[0][h][:, :], start=True, stop=False,
                        )
                    for h in range(2):
                        nc.tensor.matmul(
                            p2[m][:, h * GROUP:(h + 1) * GROUP], w2b[:, ws],
                            h1[1][h][:, :], start=False, stop=True,
                        )
                st["p2"] = p2

            def emit_tail(st):
                # residual add: channel 2p+m needs x[p]; one [128,1024] DVE op
                # per m half, then store this super-group on Sync.
                G = st["G"]
                c, si = SGS[G]
                xt_t, _ = chunks[c]
                sgsl = slice(si * SG, (si + 1) * SG)
                off = CHUNK_OFFS[c] + si * SG
                cho = [opool.tile([128, SG], bf16, tag=f"cho{m}",
                                  name=f"cho{m}_{G}") for m in range(2)]
                for m in range(2):
                    nc.vector.tensor_add(
                        cho[m][:, :], st["p2"][m][:, :], xt_t[:, sgsl])
                    nc.sync.dma_start(
                        cht[m * 128:(m + 1) * 128, off:off + SG], cho[m][:, :])
                if si == CHUNK_SIZES[c] // SG - 1 and c + PREFETCH < N_CHUNKS:
                    load_chunk(c + PREFETCH)

            prev = None
            for G in range(NSG + 1):
                cur = emit_mm1(G) if G < NSG else None
                if prev is not None:
                    emit_mm2(prev)
                if cur is not None:
                    emit_act(cur)
                if prev is not None:
                    emit_tail(prev)
                prev = cur

    _dedupe_ldweights(nc, mybir)
    _split_multiwait(nc, mybir)
    _CACHE[key] = nc
    return nc


def _host_prep(x, global_features, W1, b1, W2, b2, idxs_level, parents_idxs):
    bf = ml_dtypes.bfloat16
    x = np.ascontiguousarray(np.asarray(x, dtype=np.float32))
    G = np.asarray(global_features, dtype=np.float32)
    W1 = np.asarray(W1, dtype=np.float32)
    b1 = np.asarray(b1, dtype=np.float32)
    W2 = np.asarray(W2, dtype=np.float32)
    idxs = np.asarray(idxs_level)
    pidx = np.asarray(parents_idxs)

    if np.array_equal(idxs, np.arange(ROWS, dtype=idxs.dtype)):
        xg = x
    else:  # general gather fallback (host)
        xg = np.ascontiguousarray(x[idxs])

    xt = np.ascontiguousarray(
        xg.reshape(N_CORES, RPC, N_FEAT).transpose(0, 2, 1)).astype(bf)
    pg = G[pidx % BATCH]                              # [ROWS, 64]
    pgt = np.ascontiguousarray(
        pg.reshape(N_CORES, RPC, N_GLOBAL).transpose(0, 2, 1)).astype(bf)
    pgt = np.concatenate([pgt, pgt], axis=1)          # [8, 128, RPC]

    w1xb = W1[0:128, :]                               # [128, 256]
    # rows 0-63: W1g cols 0-127 (hid half 0); rows 64-127: cols 128-255
    w1gb = np.concatenate([W1[128:192, 0:128], W1[128:192, 128:256]], axis=0)
    # channel permutation: half m takes W2 columns m::2 (channel 2p+m at p)
    w2p = np.concatenate([W2[:, 0::2], W2[:, 1::2]], axis=1)  # [256, 256]
    wpk = np.ascontiguousarray(np.concatenate(
        [w1xb, w1gb, w2p[0:128, :], w2p[128:256, :]], axis=1)).astype(bf)
    b1c = np.ascontiguousarray(b1.reshape(2, 128).T)  # [128, 2]

    in_maps = []
    for c in range(N_CORES):
        in_maps.append({
            "xt": xt[c],
            "pgt": pgt[c],
            "wpk": wpk,
            "b1c": b1c,
        })
    return x, in_maps


# channel of cht row (m*128+p) is 2p+m; row holding channel c is (c%2)*128+c//2
_ROW_OF_CHAN = ((np.arange(HID) % 2) * 128 + np.arange(HID) // 2)


def _assemble_core(cht_arr, b2):
    b2 = np.asarray(b2, dtype=np.float32)
    A = np.asarray(cht_arr).astype(np.float32)
    projT = A[_ROW_OF_CHAN, :] + b2[:, None]           # [256 channels, RPC]
    B4 = projT.reshape(2, 128, RPC // BATCH, BATCH)    # [br, f, pp, b]
    return B4.transpose(2, 0, 3, 1).reshape(2 * RPC, N_FEAT)


def _assemble(results, b2):
    return np.concatenate(
        [_assemble_core(results[c]["cht"], b2) for c in range(N_CORES)], axis=0)


def kernel(x, global_features, W1, b1, W2, b2, idxs_level, parents_idxs,
           _trace=False, _trace_kwargs=None):
    from concourse.bass_utils import run_bass_kernel_spmd

    x_np, in_maps = _host_prep(
        x, global_features, W1, b1, W2, b2, idxs_level, parents_idxs
    )
    nc = _build_program()
    res = run_bass_kernel_spmd(
        nc, in_maps, list(range(N_CORES)),
        trace=_trace, **(_trace_kwargs or {}),
    )
    children = _assemble(res.results, b2)
    out = np.concatenate([x_np, children], axis=0)
    if _trace:
        kernel.last_result = res
    return out
